# revision 19
# baseline (speedup 1.0000x reference)
"""Trainium2 Bass kernel for nn_ATTLayer (attention pooling).

Reference computation (full input [64, 512, 1024] fp32):
    wb    = attention_w + attention_b          # [1024, 256] (b broadcast over rows)
    u_t   = tanh(inputs @ wb)                  # [64, 512, 256]
    logit = u_t @ attention_u                  # [64, 512]
    w     = softmax(logit, axis=1)             # softmax over seq
    out   = sum_s w[:, s] * inputs[:, s, :]    # [64, 1024]

Sharding: data-parallel over batch — 8 batches per core on 8 NeuronCores, no
collectives. Tiny params (wb = W + b, u) are precomputed/replicated on host.

v3 dataflow: x is uploaded ONCE, in block-transposed layout only
(xt[b, k, h_local, s], bf16) — half the HBM traffic of v1 (which also shipped
a natural-layout copy for the PE weighted sum). The weighted sum instead runs
on DVE/Pool as a fused multiply+reduce along the free (s) axis, which also
removes ~14us/core of PE work (PE is the bottleneck):

  setup: Pool-memset warmup tile (PE ramp starts at t~0, independent of any
  DMA); ACT table preloaded with a dummy tanh; wb/u consts ride the scalar
  (ACT) HWDGE ring while all xt traffic rides the SP ring.

  per local batch b (tail emitted one batch late so PE never waits on tanh):
    1. DMA xt tiles on SP ring
    2. GEMM1 (bf16, fp32 PSUM, 2 banks): p_u[a] += wb[k, a].T @ xt[k]
       over k = 8 h-chunks; single tanh [128, 1024] -> u_t^T bf16
    3. logit row [1, s]: p_l += u[a].T @ u_t^T[a]  (2 matmuls)
    4. w_row = exp(logit) on ScalarE with fused accum_out = sum (softmax
       denominator for free). No max-subtraction: |logit| <= ~20 so exp is
       finite in fp32/bf16.
    5. Pool broadcasts w_row (unnormalized) to wbc[128, s]; DVE reciprocal
       1/sum -> Pool broadcasts to rs_bc[128, 1]
    6. weighted sum per h-chunk k (5 on Pool, 3 on DVE), independent of the
       normalization: o[h_loc, b*8+k] = sum_s xt[k][h_loc, s] * wbc[h_loc, s]
       (fp32 accum via scalar_tensor_tensor's accum_out)
    7. DVE post-scales o columns by rs_bc; ONE output DMA for all batches at
       the end (o_all [128, 64] -> out[b, k*128+h_loc])

bf16 operands / fp32 accumulation end-to-end rel err ~7e-3.
"""

import numpy as np

N_CORES = 8
B_FULL = 64
B_LOC = B_FULL // N_CORES  # 8 batches per core
S = 512
H = 1024
A = 256
P = 128
NK = H // P      # 8 h-chunks
NA = A // P      # 2 a-chunks

_CACHE = {}


def _build():
    import concourse.bacc as bacc
    import concourse.mybir as mybir
    import concourse.tile as tile

    F32 = mybir.dt.float32
    BF16 = mybir.dt.bfloat16
    AF = mybir.ActivationFunctionType
    ALU = mybir.AluOpType

    nc = bacc.Bacc("TRN2", target_bir_lowering=False, debug=False)

    xt_d = nc.dram_tensor("xt", [B_LOC, NK, P, S], BF16, kind="ExternalInput").ap()
    wb_d = nc.dram_tensor("wb", [H, A], BF16, kind="ExternalInput").ap()
    u_d = nc.dram_tensor("u2", [P, NA], BF16, kind="ExternalInput").ap()
    out_d = nc.dram_tensor("out", [B_LOC, H], F32, kind="ExternalOutput").ap()

    with tile.TileContext(nc) as tc:
        with (
            tc.tile_pool(name="const", bufs=1) as cpool,
            tc.tile_pool(name="xt", bufs=6) as xtpool,
            tc.tile_pool(name="ut", bufs=4) as utpool,
            tc.tile_pool(name="sm", bufs=3) as smpool,
            tc.tile_pool(name="wbc", bufs=3) as wbcpool,
            tc.tile_pool(name="big", bufs=2) as bigpool,
            tc.tile_pool(name="scrv", bufs=2) as scrvpool,
            tc.tile_pool(name="p_u", bufs=3, space="PSUM") as p_u_pool,
            tc.tile_pool(name="p_l", bufs=2, space="PSUM") as p_l_pool,
        ):
            # ---- warmup tile (no DMA dependencies) ----
            warm_sb = cpool.tile([P, P], BF16)
            nc.gpsimd.memset(warm_sb[:], 1.0)
            # force the Q7 partition_broadcast library load to happen at t~0
            # (first use of a loadable gpsimd op pays an ~6us IRAM load)
            warm_bc = cpool.tile([P, 1], BF16)
            nc.gpsimd.partition_broadcast(warm_bc[:], warm_sb[0:1, 0:1])

            # consts ride the scalar (ACT) ring (wb split so the first gemm
            # can start as soon as its half + xt chunk 0 land); all xt
            # traffic rides the SP ring
            wb_sb = cpool.tile([P, NK * A], BF16)  # [h_local, (k a)]
            for h2 in range(2):
                nc.scalar.dma_start(
                    wb_sb[:, h2 * 4 * A : (h2 + 1) * 4 * A].rearrange(
                        "p (k a) -> p k a", k=4
                    ),
                    wb_d.rearrange("(k p) a -> p k a", p=P)[
                        :, 4 * h2 : 4 * (h2 + 1)
                    ],
                )
            u_sb = cpool.tile([P, NA], BF16)  # [a_local, a_chunk]
            nc.scalar.dma_start(u_sb[:], u_d[:])
            # ACT table preload (tanh/exp/copy share one table)
            dummy_sb = cpool.tile([1, 2], BF16)
            nc.scalar.activation(dummy_sb[:], warm_sb[0:1, 0:2], AF.Tanh)

            # output accumulator for ALL batches: col = b*NK + k
            o_all = cpool.tile([P, B_LOC * NK], F32)

            # PE warm-up from t~0 until the first wb/xt chunks land
            # (keeps the clock ramp going; ~128 cycles each)
            p_warm = p_u_pool.tile([P, 2 * S], F32, tag="p_u")
            for i in range(21):
                nc.tensor.matmul(
                    p_warm[:, 0:P], warm_sb[:], warm_sb[:],
                    start=(i == 0), stop=(i == 20),
                )

            def emit_tail(ut_sb_, xt_all_, b_):
                """logit + softmax + weighted sum for batch b_ (emitted one
                batch late so PE never waits on ACT's tanh)."""
                # ---- 3. logit row [1, s] via 2 accumulating matmuls ----
                p_l = p_l_pool.tile([1, S], F32, tag="p_l")
                for a in range(NA):
                    nc.tensor.matmul(
                        p_l[:],
                        u_sb[:, a : a + 1],
                        ut_sb_[:, a * S : (a + 1) * S],
                        start=(a == 0),
                        stop=(a == NA - 1),
                    )

                # ---- 4. w_row = exp(logit), fused softmax sum ----
                w_row = smpool.tile([1, S], BF16, tag="w_row")
                ssum = smpool.tile([1, 1], F32, tag="ssum")
                nc.scalar.activation(
                    w_row[:], p_l[:], AF.Exp, accum_out=ssum[:]
                )

                # ---- 5. broadcasts: unnormalized weights + 1/sum ----
                wbc = wbcpool.tile([P, S], BF16, tag="wbc")
                nc.gpsimd.partition_broadcast(wbc[:], w_row[:])
                rs = smpool.tile([1, 1], F32, tag="rs")
                nc.vector.reciprocal(rs[:], ssum[:])
                rs_bc = smpool.tile([P, 1], F32, tag="rs_bc")
                nc.gpsimd.partition_broadcast(rs_bc[:], rs[:])

                # ---- 6. weighted sum: one big DVE multiply (2x mode, wbc
                # repeated via stride-0 AP), then per-chunk tensor_scalar
                # accumulate (4x mode, accum_out = fp32 column sums) ----
                ocols = o_all[:, b_ * NK : (b_ + 1) * NK]
                big = bigpool.tile([P, NK * S], BF16, tag="big")
                nc.vector.tensor_tensor(
                    big[:].rearrange("p (k s) -> p k s", k=NK),
                    xt_all_[:].rearrange("p (k s) -> p k s", k=NK),
                    wbc[:].unsqueeze(1).broadcast_to([P, NK, S]),
                    op=ALU.mult,
                )
                for k in range(NK):
                    scr = scrvpool.tile([P, S], BF16, tag="scr_v")
                    nc.vector.tensor_scalar(
                        scr[:],
                        big[:, k * S : (k + 1) * S],
                        1.0,
                        None,
                        op0=ALU.mult,
                        op1=ALU.add,
                        accum_out=ocols[:, k : k + 1],
                    )

                # ---- 7. normalize the 8 accumulated columns ----
                nc.vector.tensor_scalar_mul(ocols[:], ocols[:], rs_bc[:])

            prev = None
            for b in range(B_LOC):
                # ---- 1. load xt tiles on the (otherwise idle) SP ring ----
                xt_all = xtpool.tile([P, NK * S], BF16, tag="xt")
                nsplit = 4 if b == 0 else 2
                kper = NK // nsplit
                for q in range(nsplit):
                    nc.sync.dma_start(
                        xt_all[:, q * kper * S : (q + 1) * kper * S].rearrange(
                            "p (k s) -> p k s", k=kper
                        ),
                        xt_d[b, kper * q : kper * (q + 1)].rearrange(
                            "k p s -> p k s"
                        ),
                    )
                xt_tiles = [xt_all[:, k * S : (k + 1) * S] for k in range(NK)]

                # ---- 2. GEMM1 + tanh -> u_t^T [a_local, s] ----
                # psum tile spanning 2 banks; tanh per a-chunk half so the
                # last batch's tail chain only waits on the second half
                p_u = p_u_pool.tile([P, 2 * S], F32, tag="p_u")
                ut_sb = utpool.tile([P, 2 * S], BF16, tag="ut")
                for a in range(NA):
                    for k in range(NK):
                        nc.tensor.matmul(
                            p_u[:, a * S : (a + 1) * S],
                            wb_sb[:, k * A + a * P : k * A + (a + 1) * P],
                            xt_tiles[k],
                            start=(k == 0),
                            stop=(k == NK - 1),
                        )
                    nc.scalar.activation(
                        ut_sb[:, a * S : (a + 1) * S],
                        p_u[:, a * S : (a + 1) * S],
                        AF.Tanh,
                    )

                pend = (ut_sb, xt_all, b)
                if prev is not None:
                    emit_tail(*prev)
                prev = pend

            emit_tail(*prev)

            # ---- single output DMA for all batches ----
            nc.scalar.dma_start(
                out_d.rearrange("b (k p) -> p b k", p=P),
                o_all[:].rearrange("p (b k) -> p b k", b=B_LOC),
            )

    nc.compile()
    return nc


def get_nc():
    if "nc" not in _CACHE:
        _CACHE["nc"] = _build()
    return _CACHE["nc"]


def make_in_maps(inputs, attention_w, attention_u, attention_b):
    import ml_dtypes

    bf16 = ml_dtypes.bfloat16
    x = np.asarray(inputs, dtype=np.float32).astype(bf16)
    # pre-transposed layout: xt[b, k, h_local, s] = x[b, s, k*128 + h_local]
    xt = np.ascontiguousarray(
        x.reshape(B_FULL, S, NK, P).transpose(0, 2, 3, 1)
    )
    w = np.asarray(attention_w, dtype=np.float32)
    u = np.asarray(attention_u, dtype=np.float32)
    b = np.asarray(attention_b, dtype=np.float32)
    wb = np.ascontiguousarray(w + b[None, :]).astype(bf16)
    u2 = np.zeros((P, NA), dtype=np.float32)  # [a_local, a_chunk]
    for a in range(NA):
        u2[:, a] = u[a * P : (a + 1) * P, 0]
    u2 = u2.astype(bf16)
    in_maps = []
    for c in range(N_CORES):
        in_maps.append(
            {
                "xt": xt[c * B_LOC : (c + 1) * B_LOC],
                "wb": wb,
                "u2": u2,
            }
        )
    return in_maps


def kernel(inputs, attention_w, attention_u, attention_b):
    from concourse.bass_utils import run_bass_kernel_spmd

    nc = get_nc()
    in_maps = make_in_maps(inputs, attention_w, attention_u, attention_b)
    res = run_bass_kernel_spmd(nc, in_maps, list(range(N_CORES)))
    out = np.concatenate(
        [res.results[c]["out"] for c in range(N_CORES)], axis=0
    ).astype(np.float32)
    return out


# revision 21
# speedup vs baseline: 1.4951x; 1.4951x over previous
"""Trainium2 Bass kernel for nn_ATTLayer (attention pooling).

Reference computation (full input [64, 512, 1024] fp32):
    wb    = attention_w + attention_b          # [1024, 256] (b broadcast over rows)
    u_t   = tanh(inputs @ wb)                  # [64, 512, 256]
    logit = u_t @ attention_u                  # [64, 512]
    w     = softmax(logit, axis=1)             # softmax over seq
    out   = sum_s w[:, s] * inputs[:, s, :]    # [64, 1024]

Sharding: data-parallel over batch — 8 batches per core on 8 NeuronCores, no
collectives. Tiny params (wb = W + b, u) are precomputed/replicated on host.

All heavy math runs on PE (HW-measured: back-to-back matmuls stream at full
rate; DVE reduces cost ~0.7us per [128,512] on HW, so the weighted sum stays
on PE). x is uploaded in TWO layouts (block-transposed xt for GEMM1, natural
x for the weighted sum) on separate HWDGE rings (~290 / ~220 GB/s each,
HW-measured, transfers run on DMA engines, not the issuing queue).

Per local batch b (two-level software pipelining keeps PE stall-free:
logit^T(b) is emitted after GEMM1(b+1), step7(b) after GEMM1(b+2), so the
ACT exp / Pool all-reduce / DVE normalize chain for b overlaps a full GEMM):
  1. DMA xt (SP ring), x natural (scalar ring)
  2. GEMM1: p_u[a] += wb[k,a].T @ xt[k], 8 k-chunks; tanh per a-half (ACT)
  3. logit^T [s_loc, 2t] via 8 small matmuls (ut chunk stationary, padded u)
  4. wt = exp(logit^T) on ACT with accum_out = per-partition partial sums
     (pad columns contribute exp(0)=1 each, subtracted later). No
     max-subtraction: |logit| <= ~20 so exp is finite.
  5. Pool partition_all_reduce -> every partition holds sum(w) + 512;
     DVE: subtract 512, reciprocal, wt_n = wt * rs (normalize folded into
     the step-7 stationary weights)
  6. step7: p_o[1, h] += wt_n[:, t].T @ x[t-chunk, h]  (8 matmuls)
  7. ONE ACT copy evacuates p_o -> o_all row; ONE output DMA at the end.

bf16 matmul operands / fp32 accumulation end-to-end rel err ~7e-3.
"""

import numpy as np

N_CORES = 8
B_FULL = 64
B_LOC = B_FULL // N_CORES  # 8 batches per core
S = 512
H = 1024
A = 256
P = 128
NT = S // P      # 4 s-tiles per batch
NK = H // P      # 8 h-chunks
NA = A // P      # 2 a-chunks

_CACHE = {}


def _build():
    import concourse.bacc as bacc
    import concourse.bass_isa as bass_isa
    import concourse.mybir as mybir
    import concourse.tile as tile

    F32 = mybir.dt.float32
    BF16 = mybir.dt.bfloat16
    AF = mybir.ActivationFunctionType

    nc = bacc.Bacc("TRN2", target_bir_lowering=False, debug=False)

    xt_d = nc.dram_tensor("xt", [B_LOC, NK, P, S], BF16, kind="ExternalInput").ap()
    x_d = nc.dram_tensor("x", [B_LOC, S, H], BF16, kind="ExternalInput").ap()
    wb_d = nc.dram_tensor("wb", [H, A], BF16, kind="ExternalInput").ap()
    u_d = nc.dram_tensor("u4", [P, 2 * NA], BF16, kind="ExternalInput").ap()
    out_d = nc.dram_tensor("out", [B_LOC, H], F32, kind="ExternalOutput").ap()

    with tile.TileContext(nc) as tc:
        with (
            tc.tile_pool(name="const", bufs=1) as cpool,
            tc.tile_pool(name="xt", bufs=5) as xtpool,
            tc.tile_pool(name="x", bufs=4) as xpool,
            tc.tile_pool(name="ut", bufs=3) as utpool,
            tc.tile_pool(name="sm", bufs=3) as smpool,
            tc.tile_pool(name="p_u", bufs=2, space="PSUM") as p_u_pool,
            tc.tile_pool(name="p_lt", bufs=2, space="PSUM") as p_lt_pool,
            tc.tile_pool(name="p_o", bufs=1, space="PSUM") as p_o_pool,
        ):
            # ---- warmup tile + Q7 library preload (no DMA dependencies) ----
            warm_sb = cpool.tile([P, P], BF16)
            nc.gpsimd.memset(warm_sb[:], 1.0)
            warm_f = cpool.tile([P, 1], F32)
            nc.gpsimd.memset(warm_f[:], 1.0)
            warm_ar = cpool.tile([P, 1], F32)
            nc.gpsimd.partition_all_reduce(
                warm_ar[:], warm_f[:], channels=P,
                reduce_op=bass_isa.ReduceOp.add,
            )

            # consts ride the scalar (ACT) ring, split so the first GEMM can
            # start as soon as its half + xt chunk 0 land
            wb_sb = cpool.tile([P, NK * A], BF16)  # [h_local, (k a)]
            for h2 in range(2):
                nc.scalar.dma_start(
                    wb_sb[:, h2 * 4 * A : (h2 + 1) * 4 * A].rearrange(
                        "p (k a) -> p k a", k=4
                    ),
                    wb_d.rearrange("(k p) a -> p k a", p=P)[
                        :, 4 * h2 : 4 * (h2 + 1)
                    ],
                )
            u_sb = cpool.tile([P, 2 * NA], BF16)  # [a_local, (a_chunk, 0)]
            nc.scalar.dma_start(u_sb[:], u_d[:])
            # ACT table preload (tanh/exp/copy share one table)
            dummy_sb = cpool.tile([1, 2], BF16)
            nc.scalar.activation(dummy_sb[:], warm_sb[0:1, 0:2], AF.Tanh)

            # output rows for ALL batches, partition 0: col = b*H + h
            o_all = cpool.tile([1, B_LOC * H], F32)

            # PE warm-up from t~0 until the first wb/xt chunks land
            p_warm = p_u_pool.tile([P, 2 * S], F32, tag="p_u")
            for i in range(24):
                nc.tensor.matmul(
                    p_warm[:, 0:P], warm_sb[:], warm_sb[:],
                    start=(i == 0), stop=(i == 23),
                )

            state = {}  # per-batch tiles for the two deferred stages

            def emit_logit(b_):
                ut_sb = state[b_]["ut"]
                # ---- 3. logit^T [s_loc, 2t] (pad cols stay zero) ----
                p_lt = p_lt_pool.tile([P, 2 * NT], F32, tag="p_lt")
                for t in range(NT):
                    for a in range(NA):
                        nc.tensor.matmul(
                            p_lt[:, 2 * t : 2 * t + 2],
                            ut_sb[:, a * S + t * P : a * S + (t + 1) * P],
                            u_sb[:, 2 * a : 2 * a + 2],
                            start=(a == 0),
                            stop=(a == NA - 1),
                        )
                # ---- 4. wt = exp(logit^T), partial sums per partition ----
                wt = smpool.tile([P, 2 * NT], BF16, tag="wt")
                part = smpool.tile([P, 1], F32, tag="part")
                nc.scalar.activation(
                    wt[:], p_lt[:], AF.Exp, accum_out=part[:]
                )
                # ---- 5. total sum on every partition; normalize weights ----
                tot = smpool.tile([P, 1], F32, tag="tot")
                nc.gpsimd.partition_all_reduce(
                    tot[:], part[:], channels=P,
                    reduce_op=bass_isa.ReduceOp.add,
                )
                tot2 = smpool.tile([P, 1], F32, tag="tot2")
                # pad columns contributed exp(0)=1: 4 per partition, 512 total
                nc.vector.tensor_scalar_add(tot2[:], tot[:], -512.0)
                rs = smpool.tile([P, 1], F32, tag="rs")
                nc.vector.reciprocal(rs[:], tot2[:])
                wt_n = smpool.tile([P, 2 * NT], BF16, tag="wt_n")
                nc.vector.tensor_scalar_mul(wt_n[:], wt[:], rs[:])
                state[b_]["wt_n"] = wt_n

            def emit_step7(b_):
                wt_n = state[b_]["wt_n"]
                x_sb = state[b_]["x"]
                # ---- 6. weighted sum on PE: p_o[1, h] over 4 t-chunks ----
                p_o = p_o_pool.tile([1, 2 * S], F32, tag="p_o")
                for n in range(2):
                    for t in range(NT):
                        nc.tensor.matmul(
                            p_o[:, n * S : (n + 1) * S],
                            wt_n[:, 2 * t : 2 * t + 1],
                            x_sb[:, t * H + n * S : t * H + (n + 1) * S],
                            start=(t == 0),
                            stop=(t == NT - 1),
                        )
                # ---- 7. evacuate to the output row ----
                nc.scalar.activation(
                    o_all[:, b_ * H : (b_ + 1) * H], p_o[:], AF.Copy
                )
                del state[b_]

            for b in range(B_LOC):
                # ---- 1. xt on SP ring, x natural on scalar ring ----
                xt_all = xtpool.tile([P, NK * S], BF16, tag="xt")
                nsplit = 4 if b == 0 else 2
                kper = NK // nsplit
                for q in range(nsplit):
                    nc.sync.dma_start(
                        xt_all[:, q * kper * S : (q + 1) * kper * S].rearrange(
                            "p (k s) -> p k s", k=kper
                        ),
                        xt_d[b, kper * q : kper * (q + 1)].rearrange(
                            "k p s -> p k s"
                        ),
                    )
                xt_tiles = [xt_all[:, k * S : (k + 1) * S] for k in range(NK)]
                x_sb = xpool.tile([P, NT * H], BF16, tag="x")
                nc.scalar.dma_start(
                    x_sb[:].rearrange("p (t h) -> p t h", t=NT),
                    x_d[b].rearrange("(t p) h -> p t h", p=P),
                )

                # ---- 2. GEMM1 + tanh per a-half ----
                p_u = p_u_pool.tile([P, 2 * S], F32, tag="p_u")
                ut_sb = utpool.tile([P, 2 * S], BF16, tag="ut")
                for a in range(NA):
                    for k in range(NK):
                        nc.tensor.matmul(
                            p_u[:, a * S : (a + 1) * S],
                            wb_sb[:, k * A + a * P : k * A + (a + 1) * P],
                            xt_tiles[k],
                            start=(k == 0),
                            stop=(k == NK - 1),
                        )
                    nc.scalar.activation(
                        ut_sb[:, a * S : (a + 1) * S],
                        p_u[:, a * S : (a + 1) * S],
                        AF.Tanh,
                    )
                state[b] = {"ut": ut_sb, "x": x_sb}

                # two-level deferral: logit one batch late, step7 two late
                if b >= 1:
                    emit_logit(b - 1)
                if b >= 2:
                    emit_step7(b - 2)

            emit_logit(B_LOC - 1)
            emit_step7(B_LOC - 2)
            emit_step7(B_LOC - 1)

            # ---- single output DMA for all batches (32KB contiguous) ----
            nc.scalar.dma_start(
                out_d.rearrange("b h -> (b h)").unsqueeze(0), o_all[:]
            )

    nc.compile()
    return nc


def get_nc():
    if "nc" not in _CACHE:
        _CACHE["nc"] = _build()
    return _CACHE["nc"]


def make_in_maps(inputs, attention_w, attention_u, attention_b):
    import ml_dtypes

    bf16 = ml_dtypes.bfloat16
    x = np.ascontiguousarray(
        np.asarray(inputs, dtype=np.float32).astype(bf16)
    )
    # pre-transposed layout: xt[b, k, h_local, s] = x[b, s, k*128 + h_local]
    xt = np.ascontiguousarray(
        x.reshape(B_FULL, S, NK, P).transpose(0, 2, 3, 1)
    )
    w = np.asarray(attention_w, dtype=np.float32)
    u = np.asarray(attention_u, dtype=np.float32)
    b = np.asarray(attention_b, dtype=np.float32)
    wb = np.ascontiguousarray(w + b[None, :]).astype(bf16)
    u4 = np.zeros((P, 2 * NA), dtype=np.float32)  # [a_local, (a_chunk, 0)]
    for a in range(NA):
        u4[:, 2 * a] = u[a * P : (a + 1) * P, 0]
    u4 = u4.astype(bf16)
    in_maps = []
    for c in range(N_CORES):
        in_maps.append(
            {
                "x": x[c * B_LOC : (c + 1) * B_LOC],
                "xt": xt[c * B_LOC : (c + 1) * B_LOC],
                "wb": wb,
                "u4": u4,
            }
        )
    return in_maps


def kernel(inputs, attention_w, attention_u, attention_b):
    from concourse.bass_utils import run_bass_kernel_spmd

    nc = get_nc()
    in_maps = make_in_maps(inputs, attention_w, attention_u, attention_b)
    res = run_bass_kernel_spmd(nc, in_maps, list(range(N_CORES)))
    out = np.concatenate(
        [res.results[c]["out"] for c in range(N_CORES)], axis=0
    ).astype(np.float32)
    return out


# revision 26
# speedup vs baseline: 1.6001x; 1.0702x over previous
"""Trainium2 Bass kernel for nn_ATTLayer (attention pooling).

Reference computation (full input [64, 512, 1024] fp32):
    wb    = attention_w + attention_b          # [1024, 256] (b broadcast over rows)
    u_t   = tanh(inputs @ wb)                  # [64, 512, 256]
    logit = u_t @ attention_u                  # [64, 512]
    w     = softmax(logit, axis=1)             # softmax over seq
    out   = sum_s w[:, s] * inputs[:, s, :]    # [64, 1024]

Sharding: data-parallel over batch — 8 batches per core on 8 NeuronCores, no
collectives. Tiny params (wb = W + b, u) are precomputed/replicated on host.

All heavy math runs on PE (HW-measured: back-to-back matmuls stream at full
rate; DVE reduces cost ~0.7us per [128,512] on HW, so the weighted sum stays
on PE). x is uploaded in TWO layouts (block-transposed xt for GEMM1, natural
x for the weighted sum) on separate HWDGE rings (~290 / ~220 GB/s each,
HW-measured, transfers run on DMA engines, not the issuing queue).

Per local batch b (two-level software pipelining keeps PE stall-free:
logit^T(b) is emitted after GEMM1(b+1), step7(b) after GEMM1(b+2), so the
ACT exp / Pool all-reduce / DVE normalize chain for b overlaps a full GEMM):
  1. DMA xt (SP ring), x natural (scalar ring)
  2. GEMM1: p_u[a] += wb[k,a].T @ xt[k], 8 k-chunks; tanh per a-half (ACT)
  3. logit^T [s_loc, 2t] via 8 small matmuls (ut chunk stationary, padded u)
  4. wt = exp(logit^T) on ACT with accum_out = per-partition partial sums
     (pad columns contribute exp(0)=1 each, subtracted later). No
     max-subtraction: |logit| <= ~20 so exp is finite.
  5. Pool partition_all_reduce -> every partition holds sum(w) + 512;
     DVE: subtract 512, reciprocal, wt_n = wt * rs (normalize folded into
     the step-7 stationary weights)
  6. step7: p_o[1, h] += wt_n[:, t].T @ x[t-chunk, h]  (8 matmuls)
  7. ONE ACT copy evacuates p_o -> o_all row; ONE output DMA at the end.

bf16 matmul operands / fp32 accumulation end-to-end rel err ~7e-3.
"""

import numpy as np

N_CORES = 8
B_FULL = 64
B_LOC = B_FULL // N_CORES  # 8 batches per core
S = 512
H = 1024
A = 256
P = 128
NT = S // P      # 4 s-tiles per batch
NK = H // P      # 8 h-chunks
NA = A // P      # 2 a-chunks

_CACHE = {}


def _build():
    import concourse.bacc as bacc
    import concourse.bass_isa as bass_isa
    import concourse.mybir as mybir
    import concourse.tile as tile

    F32 = mybir.dt.float32
    BF16 = mybir.dt.bfloat16
    AF = mybir.ActivationFunctionType

    nc = bacc.Bacc("TRN2", target_bir_lowering=False, debug=False)

    xt_d = nc.dram_tensor("xt", [B_LOC, NK, P, S], BF16, kind="ExternalInput").ap()
    x_d = nc.dram_tensor("x", [B_LOC, S, H], BF16, kind="ExternalInput").ap()
    wb_d = nc.dram_tensor("wb", [H, A], BF16, kind="ExternalInput").ap()
    u_d = nc.dram_tensor("u4", [P, 2 * NA], BF16, kind="ExternalInput").ap()
    out_d = nc.dram_tensor("out", [B_LOC, H], F32, kind="ExternalOutput").ap()

    with tile.TileContext(nc) as tc:
        with (
            tc.tile_pool(name="const", bufs=1) as cpool,
            tc.tile_pool(name="xt", bufs=5) as xtpool,
            tc.tile_pool(name="x", bufs=4) as xpool,
            tc.tile_pool(name="ut", bufs=3) as utpool,
            tc.tile_pool(name="sm", bufs=3) as smpool,
            tc.tile_pool(name="p_u", bufs=3, space="PSUM") as p_u_pool,
            tc.tile_pool(name="p_lt", bufs=1, space="PSUM") as p_lt_pool,
            tc.tile_pool(name="p_o", bufs=2, space="PSUM") as p_o_pool,
        ):
            # ---- warmup tile + Q7 library preload (no DMA dependencies).
            # warm_sb memset runs on DVE: the Pool queue starts with a ~6us
            # Q7 IRAM library load which would delay PE's warmup otherwise.
            warm_sb = cpool.tile([P, P], BF16)
            nc.vector.memset(warm_sb[:], 1.0)
            warm_f = cpool.tile([P, 1], F32)
            nc.gpsimd.memset(warm_f[:], 1.0)
            warm_ar = cpool.tile([P, 1], F32)
            nc.gpsimd.partition_all_reduce(
                warm_ar[:], warm_f[:], channels=P,
                reduce_op=bass_isa.ReduceOp.add,
            )

            # consts ride the scalar (ACT) ring, split so the first GEMM can
            # start as soon as its half + xt chunk 0 land
            wb_sb = cpool.tile([P, NK * A], BF16)  # [h_local, (k a)]
            for h2 in range(2):
                nc.scalar.dma_start(
                    wb_sb[:, h2 * 4 * A : (h2 + 1) * 4 * A].rearrange(
                        "p (k a) -> p k a", k=4
                    ),
                    wb_d.rearrange("(k p) a -> p k a", p=P)[
                        :, 4 * h2 : 4 * (h2 + 1)
                    ],
                )
            u_sb = cpool.tile([P, 2 * NA], BF16)  # [a_local, (a_chunk, 0)]
            nc.sync.dma_start(u_sb[:], u_d[:])
            # ACT table preload (tanh/exp/copy share one table)
            dummy_sb = cpool.tile([1, 2], BF16)
            nc.scalar.activation(dummy_sb[:], warm_sb[0:1, 0:2], AF.Tanh)

            # output rows for ALL batches, partition 0: col = b*H + h
            o_all = cpool.tile([1, B_LOC * H], F32)

            # PE warm-up from t~0 until the first wb/xt chunks land
            p_warm = p_u_pool.tile([P, S], F32, tag="p_u")
            for i in range(24):
                nc.tensor.matmul(
                    p_warm[:, 0:P], warm_sb[:], warm_sb[:],
                    start=(i == 0), stop=(i == 23),
                )

            state = {}  # per-batch tiles for the two deferred stages

            def emit_logit(b_):
                ut_sb = state[b_]["ut"]
                # ---- 3. logit^T [s_loc, 2t] (pad cols stay zero) ----
                p_lt = p_lt_pool.tile([P, 2 * NT], F32, tag="p_lt")
                for t in range(NT):
                    for a in range(NA):
                        nc.tensor.matmul(
                            p_lt[:, 2 * t : 2 * t + 2],
                            ut_sb[:, a * S + t * P : a * S + (t + 1) * P],
                            u_sb[:, 2 * a : 2 * a + 2],
                            start=(a == 0),
                            stop=(a == NA - 1),
                        )
                # ---- 4. wt = exp(logit^T), partial sums per partition ----
                wt = smpool.tile([P, 2 * NT], BF16, tag="wt")
                part = smpool.tile([P, 1], F32, tag="part")
                nc.scalar.activation(
                    wt[:], p_lt[:], AF.Exp, accum_out=part[:]
                )
                # ---- 5. total sum on every partition; normalize weights ----
                tot = smpool.tile([P, 1], F32, tag="tot")
                nc.gpsimd.partition_all_reduce(
                    tot[:], part[:], channels=P,
                    reduce_op=bass_isa.ReduceOp.add,
                )
                tot2 = smpool.tile([P, 1], F32, tag="tot2")
                # pad columns contributed exp(0)=1: 4 per partition, 512 total
                nc.vector.tensor_scalar_add(tot2[:], tot[:], -512.0)
                rs = smpool.tile([P, 1], F32, tag="rs")
                nc.vector.reciprocal(rs[:], tot2[:])
                wt_n = smpool.tile([P, 2 * NT], BF16, tag="wt_n")
                nc.vector.tensor_scalar_mul(wt_n[:], wt[:], rs[:])
                state[b_]["wt_n"] = wt_n

            def emit_step7(b_):
                wt_n = state[b_]["wt_n"]
                x_sb = state[b_]["x"]
                # ---- 6. weighted sum on PE: p_o[1, h] over 4 t-chunks ----
                p_o = p_o_pool.tile([1, 2 * S], F32, tag="p_o")
                for n in range(2):
                    for t in range(NT):
                        nc.tensor.matmul(
                            p_o[:, n * S : (n + 1) * S],
                            wt_n[:, 2 * t : 2 * t + 1],
                            x_sb[:, t * H + n * S : t * H + (n + 1) * S],
                            start=(t == 0),
                            stop=(t == NT - 1),
                        )
                # ---- 7. evacuate to the output row ----
                nc.scalar.activation(
                    o_all[:, b_ * H : (b_ + 1) * H], p_o[:], AF.Copy
                )
                del state[b_]

            for b in range(B_LOC):
                # ---- 1. xt on SP ring, x natural on scalar ring ----
                xt_all = xtpool.tile([P, NK * S], BF16, tag="xt")
                nsplit = 4 if b == 0 else 2
                kper = NK // nsplit
                for q in range(nsplit):
                    nc.sync.dma_start(
                        xt_all[:, q * kper * S : (q + 1) * kper * S].rearrange(
                            "p (k s) -> p k s", k=kper
                        ),
                        xt_d[b, kper * q : kper * (q + 1)].rearrange(
                            "k p s -> p k s"
                        ),
                    )
                xt_tiles = [xt_all[:, k * S : (k + 1) * S] for k in range(NK)]
                x_sb = xpool.tile([P, NT * H], BF16, tag="x")
                nc.scalar.dma_start(
                    x_sb[:].rearrange("p (t h) -> p t h", t=NT),
                    x_d[b].rearrange("(t p) h -> p t h", p=P),
                )

                # ---- 2. GEMM1 + tanh per a-half ----
                ut_sb = utpool.tile([P, 2 * S], BF16, tag="ut")
                for a in range(NA):
                    p_u = p_u_pool.tile([P, S], F32, tag="p_u")
                    for k in range(NK):
                        nc.tensor.matmul(
                            p_u[:],
                            wb_sb[:, k * A + a * P : k * A + (a + 1) * P],
                            xt_tiles[k],
                            start=(k == 0),
                            stop=(k == NK - 1),
                        )
                    nc.scalar.activation(
                        ut_sb[:, a * S : (a + 1) * S], p_u[:], AF.Tanh
                    )
                state[b] = {"ut": ut_sb, "x": x_sb}

                # two-level deferral: logit one batch late, step7 two late
                if b >= 1:
                    emit_logit(b - 1)
                if b >= 2:
                    emit_step7(b - 2)

            emit_logit(B_LOC - 1)
            emit_step7(B_LOC - 2)
            emit_step7(B_LOC - 1)

            # ---- single output DMA for all batches (32KB contiguous) ----
            nc.scalar.dma_start(
                out_d.rearrange("b h -> (b h)").unsqueeze(0), o_all[:]
            )

    nc.compile()
    return nc


def get_nc():
    if "nc" not in _CACHE:
        _CACHE["nc"] = _build()
    return _CACHE["nc"]


def make_in_maps(inputs, attention_w, attention_u, attention_b):
    import ml_dtypes

    bf16 = ml_dtypes.bfloat16
    x = np.ascontiguousarray(
        np.asarray(inputs, dtype=np.float32).astype(bf16)
    )
    # pre-transposed layout: xt[b, k, h_local, s] = x[b, s, k*128 + h_local]
    xt = np.ascontiguousarray(
        x.reshape(B_FULL, S, NK, P).transpose(0, 2, 3, 1)
    )
    w = np.asarray(attention_w, dtype=np.float32)
    u = np.asarray(attention_u, dtype=np.float32)
    b = np.asarray(attention_b, dtype=np.float32)
    wb = np.ascontiguousarray(w + b[None, :]).astype(bf16)
    u4 = np.zeros((P, 2 * NA), dtype=np.float32)  # [a_local, (a_chunk, 0)]
    for a in range(NA):
        u4[:, 2 * a] = u[a * P : (a + 1) * P, 0]
    u4 = u4.astype(bf16)
    in_maps = []
    for c in range(N_CORES):
        in_maps.append(
            {
                "x": x[c * B_LOC : (c + 1) * B_LOC],
                "xt": xt[c * B_LOC : (c + 1) * B_LOC],
                "wb": wb,
                "u4": u4,
            }
        )
    return in_maps


def kernel(inputs, attention_w, attention_u, attention_b):
    from concourse.bass_utils import run_bass_kernel_spmd

    nc = get_nc()
    in_maps = make_in_maps(inputs, attention_w, attention_u, attention_b)
    res = run_bass_kernel_spmd(nc, in_maps, list(range(N_CORES)))
    out = np.concatenate(
        [res.results[c]["out"] for c in range(N_CORES)], axis=0
    ).astype(np.float32)
    return out
